# revision 10
# baseline (speedup 1.0000x reference)
"""3x3 valid conv (single channel) on 8 TRN2 NeuronCores.

Strategy: shard X row-wise (512 output rows/core). Per core, five row strips:
four full strips load 128 input rows each (rows 126s..126s+127, re-reading the
2-row halo from HBM) and produce 126 output rows via 3 banded matmuls per
512-col PSUM tile; a fifth mini-strip loads rows 504..513 (10 rows) and
produces the remaining 8 output rows with K=10/M=8 matmuls, so no SBUF->SBUF
reuse copy is needed. Matmuls read the f32 tiles through float32r bitcast
views (TF32-like, 1 row/cycle once the PE p-state ramps) -- no cast pass.
PSUM drains ride the vector engine as one tensor_scalar(add bias) per
2-bank PSUM pair (~630ns/col-tile, under the PE's ~700ns/col-tile supply so
the drain never backlogs). Stores are 2MB on the scalar HWDGE ring so the
multi-MB loads on the SP ring can't head-of-line-block them; the final
strip's stores alternate onto the by-then-idle SP ring. Loads are chunked
(320K first) so the first matmul starts ~11us in.
"""

import sys

sys.path.insert(0, "/opt/trn_rl_repo")

import numpy as np
from concourse import bass, mybir
from concourse.bass_utils import run_bass_kernel_spmd
from concourse.tile import TileContext

F32 = mybir.dt.float32
F32R = mybir.dt.float32r

H, WIDTH = 4096, 8192
KH, KW = 3, 3
OH, OW = H - KH + 1, WIDTH - KW + 1
N_CORES = 8
RPC = H // N_CORES          # 512 output rows produced per core
IN_ROWS = RPC + KH - 1      # 514 input rows per core (2-row halo)
N_COL_TILES = 16            # 15 x 512 + 1 x 510 = 8190


def _split_multi_waits(nc, max_waits=1):
    # This container's walrus rejects >1 sync-wait command per instruction
    # (CoreV3 setupSyncWait). Tile attaches one wait per producing logical
    # processor to a single instruction; hoist the excess onto same-engine
    # Drain carriers inserted immediately before it.
    for fn in nc.m.functions:
        for bb in fn.blocks:
            out = []
            changed = False
            for inst in bb.instructions:
                si = inst.sync_info
                waits = list(si.on_wait) if si and si.on_wait else []
                if len(waits) > max_waits:
                    rest = waits[max_waits:]
                    for j in range(0, len(rest), max_waits):
                        carrier = mybir.InstDrain(
                            name=nc.get_next_instruction_name(), ins=[], outs=[]
                        )
                        carrier.engine = inst.engine
                        carrier.sync_info = mybir.SyncInfo(
                            on_wait=rest[j : j + max_waits], on_update=[]
                        )
                        out.append(carrier)
                    si.on_wait = waits[:max_waits]
                    changed = True
                out.append(inst)
            if changed:
                bb.instructions = out


def _build(split_waits=True):
    nc = bass.Bass()
    x = nc.declare_dram_parameter("x", [IN_ROWS, WIDTH], F32R, isOutput=False)
    bands = nc.declare_dram_parameter("bands", [128, 3 * 128], F32R, isOutput=False)
    bands4 = nc.declare_dram_parameter("bands4", [16, 3 * 8], F32R, isOutput=False)
    bias = nc.declare_dram_parameter("bias", [128, 1], F32, isOutput=False)
    y = nc.declare_dram_parameter("y", [RPC, OW], F32, isOutput=True)

    with TileContext(nc) as tc:
        with (
            tc.tile_pool(name="const", bufs=1) as cpool,
            tc.tile_pool(name="xin", bufs=4) as xpool,
            tc.tile_pool(name="stage", bufs=2) as spool,
            tc.tile_pool(name="psum", bufs=4, space="PSUM") as ppool,
        ):
            band_f = cpool.tile([128, 3 * 128], F32R)
            nc.gpsimd.dma_start(out=band_f[:], in_=bands[:])
            band4_f = cpool.tile([16, 3 * 8], F32R)
            nc.gpsimd.dma_start(out=band4_f[:], in_=bands4[:])
            bias_t = cpool.tile([128, 1], F32)
            nc.gpsimd.dma_start(out=bias_t[:], in_=bias[:])
            stage4 = cpool.tile([8, WIDTH], F32)

            x4 = None
            for s in range(4):
                r0 = 126 * s
                xt = xpool.tile([128, WIDTH], F32R, tag="xt")
                if s == 0:
                    # small first chunks so the first matmuls start early
                    # (chunk k must cover col tile k's 514-col window)
                    chunks = [(0, 640), (640, 1664), (1664, 3712), (3712, 8192)]
                else:
                    chunks = [(0, 4096), (4096, 8192)]
                for a, b in chunks:
                    nc.sync.dma_start(out=xt[:, a:b], in_=x[r0 : r0 + 128, a:b])
                if s == 3:
                    # mini-strip input: rows 504..513 -> partitions 0..9.
                    # Queued after s3's loads on the SP ring: its xin buffer
                    # (s0's) frees earliest so nothing stalls the ring.
                    x4 = xpool.tile([128, WIDTH], F32R, tag="xt")
                    nc.sync.dma_start(out=x4[0:10, :], in_=x[504:514, :])

                for g in range(2):
                    stage = spool.tile([128, 4096], F32, tag="st")
                    for p in range(4):  # 4 col-tile pairs per 2MB stage
                        ps = ppool.tile([128, 1024], F32, tag="ps")
                        for h in range(2):
                            ct = g * 8 + p * 2 + h
                            c0 = ct * 512
                            n = 512 if ct < N_COL_TILES - 1 else 510
                            for dj in range(KW):
                                nc.tensor.matmul(
                                    ps[:126, h * 512 : h * 512 + n],
                                    band_f[:, dj * 128 : dj * 128 + 126],
                                    xt[:, c0 + dj : c0 + dj + n],
                                    start=(dj == 0),
                                    stop=(dj == KW - 1),
                                )
                        pw = 1024 if g * 8 + p * 2 + 1 < N_COL_TILES - 1 else 1022
                        nc.vector.tensor_scalar_add(
                            stage[:126, p * 1024 : p * 1024 + pw],
                            ps[:126, :pw],
                            bias_t[:126, :],
                        )
                    gw = 4096 if g == 0 else 4094
                    # the final strip's stores go on the by-then-idle SP ring
                    # (its loads are done); earlier stores on the ACT ring
                    store_eng = nc.sync if (s == 3 and g == 0) else nc.scalar
                    store_eng.dma_start(
                        out=y[r0 : r0 + 126, g * 4096 : g * 4096 + gw],
                        in_=stage[:126, :gw],
                    )

            # mini-strip: outputs 504..511 from input rows 504..513
            for p in range(8):  # 8 col-tile pairs
                ps = ppool.tile([128, 1024], F32, tag="ps")
                for h in range(2):
                    ct = p * 2 + h
                    c0 = ct * 512
                    n = 512 if ct < N_COL_TILES - 1 else 510
                    for dj in range(KW):
                        nc.tensor.matmul(
                            ps[:8, h * 512 : h * 512 + n],
                            band4_f[0:10, dj * 8 : dj * 8 + 8],
                            x4[0:10, c0 + dj : c0 + dj + n],
                            start=(dj == 0),
                            stop=(dj == KW - 1),
                        )
                pw = 1024 if p < 7 else 1022
                nc.vector.tensor_scalar_add(
                    stage4[:8, p * 1024 : p * 1024 + pw],
                    ps[:8, :pw],
                    bias_t[:8, :],
                )
            nc.scalar.dma_start(out=y[504:512, :], in_=stage4[:8, :OW])

    if split_waits:
        _split_multi_waits(nc)
    return nc


_NC_CACHE = None


def _get_nc():
    global _NC_CACHE
    if _NC_CACHE is None:
        _NC_CACHE = _build()
    return _NC_CACHE


def _make_host_inputs(X, W, b):
    X = np.ascontiguousarray(np.asarray(X, dtype=np.float32))
    W = np.asarray(W, dtype=np.float32)
    b = np.asarray(b, dtype=np.float32)

    bands = np.zeros((128, 3 * 128), dtype=np.float32)
    mm = np.arange(126)
    for dj in range(KW):
        for dk in range(KH):
            # B_dj[m+dk, m] = W[dk, dj] for every output row m
            bands[mm + dk, dj * 128 + mm] = W[dk, dj]
    # mini-strip band: partition k = input local row 504+k, col m = output
    # local row 504+m; B4_dj[k, m] = W[k-m, dj]
    bands4 = np.zeros((16, 3 * 8), dtype=np.float32)
    m8 = np.arange(8)
    for dj in range(KW):
        for dk in range(KH):
            bands4[m8 + dk, dj * 8 + m8] = W[dk, dj]
    bias = np.full((128, 1), float(b[0]), dtype=np.float32)

    in_maps = []
    for i in range(N_CORES):
        r0 = i * RPC
        avail = min(IN_ROWS, H - r0)
        if avail == IN_ROWS:
            shard = X[r0 : r0 + IN_ROWS]
        else:
            shard = np.zeros((IN_ROWS, WIDTH), dtype=np.float32)
            shard[:avail] = X[r0 : r0 + avail]
        in_maps.append({"x": shard, "bands": bands, "bands4": bands4, "bias": bias})
    return in_maps


def _assemble(results):
    out = np.empty((OH, OW), dtype=np.float32)
    for i in range(N_CORES):
        r0 = i * RPC
        take = min(RPC, OH - r0)
        out[r0 : r0 + take] = results[i]["y"][:take]
    return out


def run(X, W, b, trace=False):
    nc = _get_nc()
    in_maps = _make_host_inputs(X, W, b)
    res = run_bass_kernel_spmd(nc, in_maps, list(range(N_CORES)), trace=trace)
    return _assemble(res.results), res


def kernel(X, W, b):
    out, _ = run(X, W, b)
    return out
